# revision 18
# baseline (speedup 1.0000x reference)
"""CrossCompressUnit kernel for TRN2 (8 NeuronCores, data-parallel over batch).

Math (collapsing the [B,D,D] outer product analytically):
    s1[b] = e[b,:] . w_vv      s2[b] = v[b,:] . w_ev
    s3[b] = e[b,:] . w_ve      s4[b] = v[b,:] . w_ee
    v_out[b,:] = v[b,:]*s1[b] + e[b,:]*s2[b] + b_vv
    e_out[b,:] = v[b,:]*s3[b] + e[b,:]*s4[b] + b_ee

Per-core plan (shard = 1024 rows), fp16 end-to-end (harness gate 2e-2).

  Layouts (built on host, all contiguous DMAs):
    vb/eb [128, 1024] fp16: partition p holds rows 8p..8p+7 back-to-back.
    vt/et [128, 1024] fp16: feature-major, column-permuted so PE chunk n
      column b is row 8b+n -- psum partition b == elementwise partition b.

  Per chunk-group g (chunks 2g, 2g+1), interleaved so compute streams
  behind the DMAs:
    Tensor: 4 matmuls lhsT=vt/et chunk [d,b] x w-pair [d,2] -> pg[g] fp32
    Vector: one [128,8] copy drains pg[g] -> s_sb
    Scalar: per chunk, x1 = e*s2 + b_vv, x2 = v*s3 + b_ee
            (Identity activation, scale+bias per-partition APs)
    Vector: per chunk, vo = v*s1 + x1, eo = e*s4 + x2 (stt)
  Chunk ops are [128,128]: the per-row scalars live one-per-partition.

  Queues: Scalar runs ONLY the activations (its DMA triggers would
  serialize with them); inputs ride Sync+GpSimd, stores GpSimd+Sync.
  GpSimd does no compute (it is slow and shares the SBUF port with
  Vector - measured 2.3x contention slowdown).
"""

import sys

if "/opt/trn_rl_repo" not in sys.path:
    sys.path.insert(0, "/opt/trn_rl_repo")

from contextlib import ExitStack

import numpy as np

import concourse.bass as bass
import concourse.tile as tile
from concourse import bacc
from concourse import mybir
from concourse.bass_utils import run_bass_kernel_spmd

N_CORES = 8
B, D = 8192, 128
SHARD = B // N_CORES  # 1024 rows per core
NCHUNK = SHARD // 128  # 8 chunks of 128 rows
W = SHARD

F16 = mybir.dt.float16
F32 = mybir.dt.float32
ALU = mybir.AluOpType
ACT = mybir.ActivationFunctionType

_CACHE: dict = {}


def _build_program() -> bass.Bass:
    nc = bacc.Bacc(
        "TRN2", target_bir_lowering=False, debug=False, num_devices=N_CORES
    )

    vb_d = nc.dram_tensor("vb", (128, W), F16, kind="ExternalInput").ap()
    eb_d = nc.dram_tensor("eb", (128, W), F16, kind="ExternalInput").ap()
    vt_d = nc.dram_tensor("vt", (128, W), F16, kind="ExternalInput").ap()
    et_d = nc.dram_tensor("et", (128, W), F16, kind="ExternalInput").ap()
    w2_d = nc.dram_tensor("w2", (128, 4), F16, kind="ExternalInput").ap()
    aux32_d = nc.dram_tensor("aux32", (128, 2), F32, kind="ExternalInput").ap()
    vo_d = nc.dram_tensor("v_out", (128, W), F16, kind="ExternalOutput").ap()
    eo_d = nc.dram_tensor("e_out", (128, W), F16, kind="ExternalOutput").ap()

    with tile.TileContext(nc) as tc, ExitStack() as ctx:
        const = ctx.enter_context(tc.tile_pool(name="const", bufs=1))
        io = ctx.enter_context(tc.tile_pool(name="io", bufs=1))
        sp = ctx.enter_context(tc.tile_pool(name="sp", bufs=1))
        xp = ctx.enter_context(tc.tile_pool(name="xp", bufs=4))
        ps = ctx.enter_context(tc.tile_pool(name="ps", bufs=1, space="PSUM"))

        w2 = const.tile([128, 4], F16)
        aux32 = const.tile([128, 2], F32)

        vb = io.tile([128, W], F16)
        eb = io.tile([128, W], F16)
        vt = io.tile([128, W], F16)
        et = io.tile([128, W], F16)
        vo = io.tile([128, W], F16)
        eo = io.tile([128, W], F16)
        half = W // 2
        quar = W // 4
        # Quartered leading transfers: the first matmul+act chain is gated
        # on the FIRST quarter only (trigger ~0.64us + DMA latency ~1.5us
        # each, rings pipeline in order). Sync (earliest queue) carries
        # the e-side; scalar only tiny aux32 so its queue stays clear for
        # the activation chain.
        # The scalar act chain gets ONE hoisted semaphore wait covering
        # ALL its deps, so the chain starts only when the LAST input is
        # resident. Optimize for earliest all-resident: few big transfers
        # balanced across three rings (~180GB/s each, ~640ns trigger,
        # ~1.5us latency).
        nc.sync.dma_start(w2[:], w2_d)
        nc.scalar.dma_start(aux32[:], aux32_d)
        nc.sync.dma_start(et[:], et_d)
        nc.gpsimd.dma_start(vt[:], vt_d)
        nc.scalar.dma_start(eb[:, 0:half], eb_d[:, 0:half])
        nc.gpsimd.dma_start(eb[:, half:W], eb_d[:, half:W])
        nc.scalar.dma_start(vb[:, 0:half], vb_d[:, 0:half])
        nc.sync.dma_start(vb[:, half:W], vb_d[:, half:W])

        bvv = aux32[:, 0:1]
        bee = aux32[:, 1:2]

        # Warmups: act-table load + first-op costs while inputs stream.
        wm = sp.tile([128, 4], F32)
        nc.vector.tensor_copy(wm[:, 0:1], w2[:, 0:1])
        nc.scalar.activation(wm[:, 2:3], w2[:, 0:1], ACT.Identity)

        # s layout, group g = chunks (2g, 2g+1), r = chunk within group:
        #   col 8g+2r = s2, 8g+2r+1 = s4, 8g+4+2r = s1, 8g+4+2r+1 = s3
        pg = [ps.tile([128, 8], F32, name=f"pg{g}") for g in range(4)]
        s_sb = sp.tile([128, 4 * NCHUNK], F32)

        for g in range(4):
            for r in range(2):
                n = 2 * g + r
                c = slice(n * D, (n + 1) * D)
                nc.tensor.matmul(pg[g][:, 2 * r : 2 * r + 2],
                                 lhsT=vt[:, c], rhs=w2[:, 0:2],
                                 start=True, stop=True)
                nc.tensor.matmul(pg[g][:, 4 + 2 * r : 4 + 2 * r + 2],
                                 lhsT=et[:, c], rhs=w2[:, 2:4],
                                 start=True, stop=True)
            nc.vector.tensor_copy(s_sb[:, 8 * g : 8 * g + 8], pg[g][:])
            for r in range(2):
                n = 2 * g + r
                c = slice(n * D, (n + 1) * D)
                s2c = s_sb[:, 8 * g + 2 * r : 8 * g + 2 * r + 1]
                s4c = s_sb[:, 8 * g + 2 * r + 1 : 8 * g + 2 * r + 2]
                s1c = s_sb[:, 8 * g + 4 + 2 * r : 8 * g + 4 + 2 * r + 1]
                s3c = s_sb[:, 8 * g + 4 + 2 * r + 1 : 8 * g + 4 + 2 * r + 2]
                x1 = xp.tile([128, D], F16)
                nc.scalar.activation(x1[:], eb[:, c], ACT.Identity,
                                     bias=bvv, scale=s2c)
                nc.vector.scalar_tensor_tensor(
                    vo[:, c], vb[:, c], s1c, x1[:], ALU.mult, ALU.add
                )
                x2 = xp.tile([128, D], F16)
                if n < 6:
                    nc.scalar.activation(x2[:], vb[:, c], ACT.Identity,
                                         bias=bee, scale=s3c)
                else:
                    # rebalance the tail: Vector's tensor_scalar (4x mode)
                    # takes over x2 so the Scalar act chain ends sooner.
                    nc.vector.tensor_scalar(x2[:], vb[:, c], s3c, bee,
                                            ALU.mult, ALU.add)
                nc.vector.scalar_tensor_tensor(
                    eo[:, c], eb[:, c], s4c, x2[:], ALU.mult, ALU.add
                )
            if g == 1:
                nc.gpsimd.dma_start(vo_d[:, 0:half], vo[:, 0:half])
                nc.gpsimd.dma_start(eo_d[:, 0:half], eo[:, 0:half])
            elif g == 2:
                nc.gpsimd.dma_start(vo_d[:, half : half + quar],
                                    vo[:, half : half + quar])
                nc.gpsimd.dma_start(eo_d[:, half : half + quar],
                                    eo[:, half : half + quar])

        # tiny final quarters on two parallel idle rings -> short tail.
        nc.scalar.dma_start(vo_d[:, W - quar : W], vo[:, W - quar : W])
        nc.sync.dma_start(eo_d[:, W - quar : W], eo[:, W - quar : W])

    nc.compile()
    return nc


def _get_program() -> bass.Bass:
    if "nc" not in _CACHE:
        _CACHE["nc"] = _build_program()
    return _CACHE["nc"]


def kernel(v, e, w_vv, b_vv, w_ev, w_ve, w_ee, b_ee, _trace=False):
    v = np.ascontiguousarray(v, dtype=np.float32)
    e = np.ascontiguousarray(e, dtype=np.float32)
    assert v.shape == (B, D) and e.shape == (B, D)

    w2 = np.empty((128, 4), dtype=np.float16)
    w2[:, 0] = np.asarray(w_ev, dtype=np.float16)
    w2[:, 1] = np.asarray(w_ee, dtype=np.float16)
    w2[:, 2] = np.asarray(w_vv, dtype=np.float16)
    w2[:, 3] = np.asarray(w_ve, dtype=np.float16)
    aux32 = np.empty((128, 2), dtype=np.float32)
    aux32[:, 0] = np.float32(np.asarray(b_vv).reshape(-1)[0])
    aux32[:, 1] = np.float32(np.asarray(b_ee).reshape(-1)[0])

    v16 = v.astype(np.float16)
    e16 = e.astype(np.float16)

    in_maps = []
    for i in range(N_CORES):
        sl = slice(i * SHARD, (i + 1) * SHARD)
        in_maps.append(
            {
                # vb[p, n*D+d] = v[8p+n, d]; vt[d, n*D+b] = v[8b+n, d] so the
                # PE's psum partition b for chunk n is the same row the
                # elementwise phase sees at partition b, chunk n.
                "vb": v16[sl].reshape(128, W),
                "eb": e16[sl].reshape(128, W),
                "vt": v16[sl].reshape(128, NCHUNK, D).transpose(2, 1, 0).reshape(128, W),
                "et": e16[sl].reshape(128, NCHUNK, D).transpose(2, 1, 0).reshape(128, W),
                "w2": w2,
                "aux32": aux32,
            }
        )

    nc = _get_program()
    try:
        res = run_bass_kernel_spmd(
            nc, in_maps, core_ids=list(range(N_CORES)), trace=_trace
        )
    except Exception:
        # The first execution after a fresh NEFF load occasionally reports
        # the device unrecoverable; a retry on a re-initialized client works.
        import time as _time

        _time.sleep(2.0)
        res = run_bass_kernel_spmd(
            nc, in_maps, core_ids=list(range(N_CORES)), trace=_trace
        )

    v_out = np.concatenate(
        [np.asarray(r["v_out"]).astype(np.float32).reshape(SHARD, D)
         for r in res.results],
        axis=0,
    )
    e_out = np.concatenate(
        [np.asarray(r["e_out"]).astype(np.float32).reshape(SHARD, D)
         for r in res.results],
        axis=0,
    )
    if _trace:
        _CACHE["last_results"] = res
    return (v_out, e_out)
